# revision 18
# baseline (speedup 1.0000x reference)
"""Trainium2 Bass kernel for nn_MultiHeadAttention_60378650247490.

Sharding: 8 cores = 2 batches x 4 head-groups (4 heads each).
Core c handles batch b = c // 4, heads [h0, h0+4) with h0 = (c % 4) * 4.

Per-core device program (fp16 data path, f32 PSUM/softmax):
  - LayerNorm stats of query[b] computed from the transposed activation
    qT [d, q] via ones-matmuls (sum, sum-of-squares) on the PE.
  - Q/K projections produce per-head transposed tiles Qaug/Kaug [65, L]:
    rows 0-63 are the head dims; row 64 is an augmentation row
    (Q: ones, K: mask-folded lex vector) so "logits += lex_mask" and the
    masking ride the QK matmul for free.  The LN mean/rstd correction and
    the q-bias fold in as 2 extra contraction rows.
  - Main loop over (head, q-tile of 128):
      bias  = pos_bias (DMA) + postag_bias (accum-DMA, adds during DMA)
      u     = max(QK_psum, -80) + bias     (one fused DVE op, f32)
      attn16= exp(u) -> fp16, rowsum via accum_out (ACT, f32 sum)
      attn16 *= 1/rowsum                   (DVE per-partition mul)
      cast-DMA attn16 -> f32 HBM (gpsimd); XBAR DMA-transpose attn16
      (scalar-engine DGE) -> aT; AV matmul fp16; out-projection fp16.
  - Host only marshals: transposes query[b], slices/folds weights to fp16,
    sums the 8 partial outputs, adds bo + residual.
"""

import numpy as np

B, L, D, H = 2, 1024, 1024, 16
NCORES = 8
NH = 4            # heads per core
dk = D // H       # 64
P = 128
DC = 8            # d-chunks of 128 (contraction for projections)
KC = 8            # k-chunks of 128
HD = NH * dk      # 256 head-dims per core
EPS = 1e-6
SCALE = float(dk) ** 0.5
NEG = -3000.0     # large negative, safe for fp16 and exp table

_BUILT = None


def build_nc():
    import concourse.bass as bass
    import concourse.bacc as bacc
    import concourse.tile as tile
    import concourse.mybir as mybir
    from concourse.masks import make_identity
    from contextlib import ExitStack

    dt = mybir.dt
    f32 = dt.float32
    f16 = dt.float16
    AF = mybir.ActivationFunctionType
    ALU = mybir.AluOpType

    nc = bacc.Bacc("TRN2", target_bir_lowering=False, debug=False,
                   num_devices=NCORES)

    # ---- per-core I/O ----
    qT = nc.dram_tensor("qT", [D, L], f16, kind="ExternalInput")
    pos = nc.dram_tensor("pos", [NH, L, L], f32, kind="ExternalInput")
    postag = nc.dram_tensor("postag", [NH, L, L], f32, kind="ExternalInput")
    wq = nc.dram_tensor("wq", [D, HD], f16, kind="ExternalInput")
    wk = nc.dram_tensor("wk", [D, HD], f16, kind="ExternalInput")
    wv = nc.dram_tensor("wv", [D, HD], f16, kind="ExternalInput")
    wo = nc.dram_tensor("wo", [HD, D], f16, kind="ExternalInput")
    qaug_w = nc.dram_tensor("qaug_w", [2, HD], f16, kind="ExternalInput")
    kv_b = nc.dram_tensor("kv_b", [2, HD], f16, kind="ExternalInput")
    lexm = nc.dram_tensor("lexm", [1, L], f16, kind="ExternalInput")
    onesv = nc.dram_tensor("onesv", [1, L], f16, kind="ExternalInput")
    qrow = nc.dram_tensor("qrow", [L, D], f16, kind="ExternalInput")
    attn_out = nc.dram_tensor("attn_out", [NH, L, L], f16, kind="ExternalOutput")
    out_part = nc.dram_tensor("out_part", [L, D], f32, kind="ExternalOutput")

    with tile.TileContext(nc) as tc:
        with ExitStack() as ctx:
            singles = ctx.enter_context(tc.tile_pool(name="singles", bufs=1))

            ident16 = singles.tile([P, P], f16)
            make_identity(nc, ident16)
            ones_row = singles.tile([1, L], f16)
            nc.sync.dma_start(ones_row, onesv[:])

            wq_sb = singles.tile([P, DC, HD], f16)
            nc.sync.dma_start(wq_sb, wq[:].rearrange("(c p) m -> p c m", p=P))
            wk_sb = singles.tile([P, DC, HD], f16)
            nc.sync.dma_start(wk_sb, wk[:].rearrange("(c p) m -> p c m", p=P))
            wv_sb = singles.tile([P, DC, HD], f16)
            nc.sync.dma_start(wv_sb, wv[:].rearrange("(c p) m -> p c m", p=P))
            wo_sb = singles.tile([P, 2, D], f16)
            nc.sync.dma_start(wo_sb, wo[:].rearrange("(c p) m -> p c m", p=P))
            qaug_sb = singles.tile([2, HD], f16)
            nc.sync.dma_start(qaug_sb, qaug_w[:])
            kvb_part = singles.tile([P, 2], f32)
            nc.gpsimd.dma_start(
                kvb_part, kv_b[0:1, :].rearrange("o (h d) -> d (o h)", d=P))
            bv_sb = singles.tile([1, HD], f16)
            nc.sync.dma_start(bv_sb, kv_b[1:2, :])

            QaugT = singles.tile([65, NH, L], f16)
            KaugT = singles.tile([65, NH, L], f16)
            V_sb = singles.tile([P, KC, HD], f16)
            OT128 = singles.tile([P, 2, L], f16)

            # augmentation rows: Q row 64 = 1.0, K row 64 = lex-masked vector
            for h in range(NH):
                nc.sync.dma_start(QaugT[64:65, h, :], onesv[:])
                nc.sync.dma_start(KaugT[64:65, h, :], lexm[:])

            # ---------------- phase 1+2: LN stats + projections -------------
            with ExitStack() as pctx:
                ph1 = pctx.enter_context(tc.tile_pool(name="ph1", bufs=2))
                ph2ps = pctx.enter_context(
                    tc.tile_pool(name="ph2ps", bufs=3, space="PSUM"))
                dramp = pctx.enter_context(
                    tc.tile_pool(name="dramp", bufs=1, space="DRAM"))

                qt_sb = ph1.tile([P, DC, L], f16, bufs=1)
                nc.sync.dma_start(
                    qt_sb, qT[:].rearrange("(c p) q -> p c q", p=P))

                with nc.named_scope("ln_stats"):
                    eps_p = ph1.tile([P, 1], f32, bufs=1)
                    nc.vector.memset(eps_p, EPS)
                    stats_qp = ph1.tile([P, 8, 2], f32, bufs=1)
                    for qt in range(8):
                        qr = ph1.tile([P, D], f16, tag="qr")
                        nc.sync.dma_start(qr, qrow[qt * P:(qt + 1) * P, :])
                        bst = ph1.tile([P, 2, 6], f32, tag="bst")
                        for sg in range(2):
                            nc.vector.bn_stats(
                                bst[:, sg, :], qr[:, sg * 512:(sg + 1) * 512])
                        nc.vector.bn_aggr(stats_qp[:, qt, :], bst)

                    negm_qp = ph1.tile([P, 8], f32, bufs=1)
                    std_qp = ph1.tile([P, 8], f16, bufs=1)
                    rstd_qp = ph1.tile([P, 8], f32, bufs=1)
                    nc.vector.tensor_scalar_mul(negm_qp, stats_qp[:, :, 0],
                                                -1.0)
                    nc.scalar.activation(std_qp, stats_qp[:, :, 1], AF.Sqrt,
                                         bias=eps_p)
                    nc.vector.reciprocal(rstd_qp, std_qp)

                    augR = ph1.tile([2, L], f16, bufs=1)
                    rbc = ph1.tile([P, L], f32, bufs=1)
                    # scatter [P, 8] partition-layout stats to q-ordered DRAM
                    # (element (p, qt) -> flat q = qt*128+p), then read back
                    negm16 = ph1.tile([P, 8], f16, bufs=1)
                    nc.vector.tensor_copy(negm16, negm_qp)
                    s_negm = dramp.tile([L], f16)
                    s_std = dramp.tile([L], f16)
                    s_rstd = dramp.tile([L], f32)
                    nc.sync.dma_start(
                        s_negm[None, :].rearrange("o (t p) -> o p t", p=P),
                        negm16)
                    nc.sync.dma_start(
                        s_std[None, :].rearrange("o (t p) -> o p t", p=P),
                        std_qp)
                    nc.sync.dma_start(
                        s_rstd[None, :].rearrange("o (t p) -> o p t", p=P),
                        rstd_qp)
                    nc.sync.dma_start(augR[0:1, :], s_negm[None, :])
                    nc.sync.dma_start(augR[1:2, :], s_std[None, :])
                    nc.sync.dma_start(rbc, s_rstd[None, :].to_broadcast((P, L)))

                with nc.named_scope("proj_qk"):
                    for hp in range(2):
                        hpc = slice(hp * P, (hp + 1) * P)
                        for qb in range(2):
                            qs = slice(qb * 512, (qb + 1) * 512)
                            kps = ph2ps.tile([P, 512], f32, tag="proj")
                            for c in range(DC):
                                nc.tensor.matmul(
                                    kps, wk_sb[:, c, hpc], qt_sb[:, c, qs],
                                    start=(c == 0), stop=(c == DC - 1))
                            nc.vector.tensor_scalar_add(
                                KaugT[0:64, 2 * hp, qs], kps[0:64, :],
                                kvb_part[0:64, hp:hp + 1])
                            nc.vector.tensor_scalar_add(
                                KaugT[0:64, 2 * hp + 1, qs], kps[64:P, :],
                                kvb_part[64:P, hp:hp + 1])

                            qps = ph2ps.tile([P, 512], f32, tag="proj")
                            for c in range(DC):
                                nc.tensor.matmul(
                                    qps, wq_sb[:, c, hpc], qt_sb[:, c, qs],
                                    start=(c == 0), stop=False)
                            nc.tensor.matmul(
                                qps, qaug_sb[0:2, hpc], augR[0:2, qs],
                                start=False, stop=True)
                            nc.vector.tensor_mul(QaugT[0:64, 2 * hp, qs],
                                                 qps[0:64, :], rbc[0:64, qs])
                            nc.vector.tensor_mul(QaugT[0:64, 2 * hp + 1, qs],
                                                 qps[64:P, :], rbc[64:P, qs])

                with nc.named_scope("proj_v"):
                    for kc in range(KC):
                        ks = slice(kc * P, (kc + 1) * P)
                        vps = ph2ps.tile([P, HD], f32, tag="vproj")
                        for c in range(DC):
                            nc.tensor.matmul(
                                vps, qt_sb[:, c, ks], wv_sb[:, c, :],
                                start=(c == 0), stop=False)
                        nc.tensor.matmul(
                            vps, ones_row[0:1, 0:P], bv_sb[0:1, :],
                            start=False, stop=True)
                        nc.vector.tensor_copy(V_sb[:, kc, :], vps)

            # ---------------- phase 3: attention main loop -------------------
            with ExitStack() as mctx:
                mn = mctx.enter_context(tc.tile_pool(name="mn", bufs=3))
                biasp = mctx.enter_context(tc.tile_pool(name="biasp", bufs=6))

                mps = mctx.enter_context(
                    tc.tile_pool(name="mps", bufs=1, space="PSUM"))

                aT = None
                for h in range(NH):
                    for qt in range(8):
                        qs128 = slice(qt * P, (qt + 1) * P)
                        qb, qi = divmod(qt, 4)

                        bias_sb = biasp.tile([P, L], f32, tag="bias")
                        nc.sync.dma_start(bias_sb, pos[h, qs128, :])
                        nc.gpsimd.dma_start(bias_sb, postag[h, qs128, :],
                                            accum_op=ALU.add)

                        qk = mps.tile([P, L], f32, tag="qk", bufs=2)
                        for s2 in range(2):
                            ss = slice(s2 * 512, (s2 + 1) * 512)
                            nc.tensor.matmul(
                                qk[:, ss], QaugT[0:65, h, qs128],
                                KaugT[0:65, h, ss], start=True, stop=True)

                        u = mn.tile([P, L], f32, tag="u")
                        nc.vector.scalar_tensor_tensor(
                            u, qk, -80.0, bias_sb,
                            op0=ALU.max, op1=ALU.add)

                        attn16 = mn.tile([P, L], f16, tag="attn")
                        rowsum = mn.tile([P, 1], f32, tag="rs")
                        nc.scalar.activation(attn16, u, AF.Exp,
                                             accum_out=rowsum)
                        rrec = mn.tile([P, 1], f32, tag="rr")
                        nc.vector.reciprocal(rrec, rowsum)
                        nc.vector.tensor_scalar_mul(attn16, attn16, rrec)
                        nc.sync.dma_start(attn_out[h, qs128, :], attn16)

                        if qi == 0:
                            aT = mn.tile([P, KC, 512], f16, tag="aT", bufs=2)
                        for half in range(2):
                            t4 = mps.tile([P, 4, P], f16, tag="t4", bufs=2)
                            for j in range(4):
                                c = half * 4 + j
                                nc.tensor.transpose(
                                    t4[:, j, :],
                                    attn16[:, c * P:(c + 1) * P], ident16)
                            nc.vector.tensor_copy(
                                aT[:, half * 4:half * 4 + 4,
                                   qi * P:(qi + 1) * P], t4)

                        if qi == 3:
                            qsb = slice(qb * 512, (qb + 1) * 512)
                            otps = mps.tile([64, 512], f32, tag="ot", bufs=1)
                            for c in range(KC):
                                nc.tensor.matmul(
                                    otps, V_sb[:, c, h * dk:(h + 1) * dk],
                                    aT[:, c, :],
                                    start=(c == 0), stop=(c == KC - 1))
                            hb = (h % 2) * 64
                            nc.vector.tensor_copy(
                                OT128[hb:hb + 64, h // 2, qsb], otps)

                            if h == NH - 1:
                                with nc.named_scope("out_proj"):
                                    for qc in range(qb * 4, qb * 4 + 4):
                                        qcs = slice(qc * P, (qc + 1) * P)
                                        for dmb in range(2):
                                            ds_ = slice(dmb * 512,
                                                        (dmb + 1) * 512)
                                            ops_ = mps.tile([P, 512], f32,
                                                            tag="op", bufs=1)
                                            for c in range(2):
                                                nc.tensor.matmul(
                                                    ops_,
                                                    OT128[:, c, qcs],
                                                    wo_sb[:, c, ds_],
                                                    start=(c == 0),
                                                    stop=(c == 1))
                                            osb = mn.tile([P, 512], f32,
                                                          tag="osb")
                                            nc.vector.tensor_copy(osb, ops_)
                                            nc.sync.dma_start(
                                                out_part[qcs, ds_], osb)
    nc.compile()
    return nc


def _get_nc():
    global _BUILT
    if _BUILT is None:
        _BUILT = build_nc()
    return _BUILT


def make_in_maps(inputs):
    """Host-side marshalling: shard the full inputs into 8 per-core maps."""
    f = np.float32
    h16 = np.float16
    q = np.asarray(inputs["query"], f)
    pos_bias = np.asarray(inputs["pos_bias"], f)
    postag_bias = np.asarray(inputs["postag_bias"], f)
    lex_mask = np.asarray(inputs["lex_mask"], f)
    mask = np.asarray(inputs["mask"])
    Wq = np.asarray(inputs["Wq"], f)
    bq = np.asarray(inputs["bq"], f)
    Wk = np.asarray(inputs["Wk"], f)
    bk = np.asarray(inputs["bk"], f)
    Wv = np.asarray(inputs["Wv"], f)
    bv = np.asarray(inputs["bv"], f)
    Wo = np.asarray(inputs["Wo"], f)
    gamma = np.asarray(inputs["gamma"], f)
    beta = np.asarray(inputs["beta"], f)

    WqT_g = (Wq.T * gamma[:, None]) / SCALE          # [d, out], folds LN gain
    bq_eff = (bq + beta @ Wq.T) / SCALE              # folds LN shift
    colsum = WqT_g.sum(axis=0)                       # mean-correction weights
    WkT = np.ascontiguousarray(Wk.T)
    WvT = np.ascontiguousarray(Wv.T)
    WoT = np.ascontiguousarray(Wo.T)

    in_maps = []
    for core in range(NCORES):
        b = core // 4
        g = core % 4
        h0 = g * NH
        cols = slice(g * HD, (g + 1) * HD)
        lexm_b = np.where(mask[b] == 0, NEG, lex_mask[b]).astype(f)
        in_maps.append({
            "qT": np.ascontiguousarray(q[b].T.astype(h16)),
            "qrow": np.ascontiguousarray(q[b].astype(h16)),
            "pos": np.ascontiguousarray(pos_bias[h0:h0 + NH]),
            "postag": np.ascontiguousarray(postag_bias[b, h0:h0 + NH]),
            "wq": np.ascontiguousarray(WqT_g[:, cols].astype(h16)),
            "wk": np.ascontiguousarray(WkT[:, cols].astype(h16)),
            "wv": np.ascontiguousarray(WvT[:, cols].astype(h16)),
            "wo": np.ascontiguousarray(WoT[cols, :].astype(h16)),
            "qaug_w": np.ascontiguousarray(
                np.stack([colsum[cols], bq_eff[cols]]).astype(h16)),
            "kv_b": np.ascontiguousarray(
                np.stack([bk[cols], bv[cols]]).astype(h16)),
            "lexm": np.ascontiguousarray(lexm_b[None, :].astype(h16)),
            "onesv": np.ones((1, L), h16),
        })
    return in_maps


def gather_outputs(inputs, results):
    f = np.float32
    q = np.asarray(inputs["query"], f)
    bo = np.asarray(inputs["bo"], f)
    attn = np.empty((B, H, L, L), f)
    out = np.empty((B, L, D), f)
    for core in range(NCORES):
        b = core // 4
        h0 = (core % 4) * NH
        attn[b, h0:h0 + NH] = results[core]["attn_out"].astype(f)
    for b in range(B):
        acc = results[b * 4]["out_part"].astype(f).copy()
        for c in range(b * 4 + 1, b * 4 + 4):
            acc += results[c]["out_part"]
        out[b] = acc + bo[None, :] + q[b]
    return out, attn


def run(inputs, **spmd_kwargs):
    """Run on hardware; returns ((out, attn), BassKernelResults)."""
    from concourse.bass_utils import run_bass_kernel_spmd
    nc = _get_nc()
    in_maps = make_in_maps(inputs)
    res = run_bass_kernel_spmd(nc, in_maps, core_ids=list(range(NCORES)),
                               **spmd_kwargs)
    return gather_outputs(inputs, res.results), res


def kernel(**inputs):
    outputs, _ = run(inputs)
    return outputs


# revision 20
# speedup vs baseline: 1.1602x; 1.1602x over previous
"""Trainium2 Bass kernel for nn_MultiHeadAttention_60378650247490.

Sharding: 8 cores = 2 batches x 4 head-groups (4 heads each).
Core c handles batch b = c // 4, heads [h0, h0+4) with h0 = (c % 4) * 4.

Per-core device program (fp16 data path, f32 PSUM/softmax):
  - LayerNorm stats of query[b] computed from the transposed activation
    qT [d, q] via ones-matmuls (sum, sum-of-squares) on the PE.
  - Q/K projections produce per-head transposed tiles Qaug/Kaug [65, L]:
    rows 0-63 are the head dims; row 64 is an augmentation row
    (Q: ones, K: mask-folded lex vector) so "logits += lex_mask" and the
    masking ride the QK matmul for free.  The LN mean/rstd correction and
    the q-bias fold in as 2 extra contraction rows.
  - Main loop over (head, q-tile of 128):
      bias  = pos_bias (DMA) + postag_bias (accum-DMA, adds during DMA)
      u     = max(QK_psum, -80) + bias     (one fused DVE op, f32)
      attn16= exp(u) -> fp16, rowsum via accum_out (ACT, f32 sum)
      attn16 *= 1/rowsum                   (DVE per-partition mul)
      cast-DMA attn16 -> f32 HBM (gpsimd); XBAR DMA-transpose attn16
      (scalar-engine DGE) -> aT; AV matmul fp16; out-projection fp16.
  - Host only marshals: transposes query[b], slices/folds weights to fp16,
    sums the 8 partial outputs, adds bo + residual.
"""

import numpy as np

B, L, D, H = 2, 1024, 1024, 16
NCORES = 8
NH = 4            # heads per core
dk = D // H       # 64
P = 128
DC = 8            # d-chunks of 128 (contraction for projections)
KC = 8            # k-chunks of 128
HD = NH * dk      # 256 head-dims per core
EPS = 1e-6
SCALE = float(dk) ** 0.5
NEG = -3000.0     # large negative, safe for fp16 and exp table

_BUILT = None


def build_nc():
    import concourse.bass as bass
    import concourse.bacc as bacc
    import concourse.tile as tile
    import concourse.mybir as mybir
    from concourse.masks import make_identity
    from contextlib import ExitStack

    dt = mybir.dt
    f32 = dt.float32
    f16 = dt.float16
    AF = mybir.ActivationFunctionType
    ALU = mybir.AluOpType

    nc = bacc.Bacc("TRN2", target_bir_lowering=False, debug=False,
                   num_devices=NCORES)

    # ---- per-core I/O ----
    qT = nc.dram_tensor("qT", [D, L], f16, kind="ExternalInput")
    pos = nc.dram_tensor("pos", [NH, L, L], f16, kind="ExternalInput")
    postag = nc.dram_tensor("postag", [NH, L, L], f16, kind="ExternalInput")
    wq = nc.dram_tensor("wq", [D, HD], f16, kind="ExternalInput")
    wk = nc.dram_tensor("wk", [D, HD], f16, kind="ExternalInput")
    wv = nc.dram_tensor("wv", [D, HD], f16, kind="ExternalInput")
    wo = nc.dram_tensor("wo", [HD, D], f16, kind="ExternalInput")
    qaug_w = nc.dram_tensor("qaug_w", [2, HD], f16, kind="ExternalInput")
    kv_b = nc.dram_tensor("kv_b", [2, HD], f16, kind="ExternalInput")
    lexm = nc.dram_tensor("lexm", [1, L], f16, kind="ExternalInput")
    onesv = nc.dram_tensor("onesv", [1, L], f16, kind="ExternalInput")
    qrow = nc.dram_tensor("qrow", [L, D], f16, kind="ExternalInput")
    attn_out = nc.dram_tensor("attn_out", [NH, L, L], f16, kind="ExternalOutput")
    out_part = nc.dram_tensor("out_part", [L, D], f16, kind="ExternalOutput")

    with tile.TileContext(nc) as tc:
        with ExitStack() as ctx:
            singles = ctx.enter_context(tc.tile_pool(name="singles", bufs=1))

            ident16 = singles.tile([P, P], f16)
            make_identity(nc, ident16)
            ones_row = singles.tile([1, L], f16)
            nc.sync.dma_start(ones_row, onesv[:])

            wq_sb = singles.tile([P, DC, HD], f16)
            nc.sync.dma_start(wq_sb, wq[:].rearrange("(c p) m -> p c m", p=P))
            wk_sb = singles.tile([P, DC, HD], f16)
            nc.sync.dma_start(wk_sb, wk[:].rearrange("(c p) m -> p c m", p=P))
            wv_sb = singles.tile([P, DC, HD], f16)
            nc.sync.dma_start(wv_sb, wv[:].rearrange("(c p) m -> p c m", p=P))
            wo_sb = singles.tile([P, 2, D], f16)
            nc.sync.dma_start(wo_sb, wo[:].rearrange("(c p) m -> p c m", p=P))
            qaug_sb = singles.tile([2, HD], f16)
            nc.sync.dma_start(qaug_sb, qaug_w[:])
            kvb_part = singles.tile([P, 2], f32)
            nc.gpsimd.dma_start(
                kvb_part, kv_b[0:1, :].rearrange("o (h d) -> d (o h)", d=P))
            bv_sb = singles.tile([1, HD], f16)
            nc.sync.dma_start(bv_sb, kv_b[1:2, :])

            QaugT = singles.tile([65, NH, L], f16)
            KaugT = singles.tile([65, NH, L], f16)
            V_sb = singles.tile([P, KC, HD], f16)
            OT128 = singles.tile([P, 2, L], f16)

            # augmentation rows: Q row 64 = 1.0, K row 64 = lex-masked vector
            for h in range(NH):
                nc.sync.dma_start(QaugT[64:65, h, :], onesv[:])
                nc.sync.dma_start(KaugT[64:65, h, :], lexm[:])

            # ---------------- phase 1+2: LN stats + projections -------------
            with ExitStack() as pctx:
                ph1 = pctx.enter_context(tc.tile_pool(name="ph1", bufs=2))
                ph2ps = pctx.enter_context(
                    tc.tile_pool(name="ph2ps", bufs=3, space="PSUM"))
                dramp = pctx.enter_context(
                    tc.tile_pool(name="dramp", bufs=1, space="DRAM"))

                qt_sb = ph1.tile([P, DC, L], f16, bufs=1)
                nc.sync.dma_start(
                    qt_sb, qT[:].rearrange("(c p) q -> p c q", p=P))

                with nc.named_scope("ln_stats"):
                    eps_p = ph1.tile([P, 1], f32, bufs=1)
                    nc.vector.memset(eps_p, EPS)
                    stats_qp = ph1.tile([P, 8, 2], f32, bufs=1)
                    for qt in range(8):
                        qr = ph1.tile([P, D], f16, tag="qr")
                        nc.sync.dma_start(qr, qrow[qt * P:(qt + 1) * P, :])
                        bst = ph1.tile([P, 2, 6], f32, tag="bst")
                        for sg in range(2):
                            nc.vector.bn_stats(
                                bst[:, sg, :], qr[:, sg * 512:(sg + 1) * 512])
                        nc.vector.bn_aggr(stats_qp[:, qt, :], bst)

                    negm_qp = ph1.tile([P, 8], f32, bufs=1)
                    std_qp = ph1.tile([P, 8], f16, bufs=1)
                    rstd_qp = ph1.tile([P, 8], f32, bufs=1)
                    nc.vector.tensor_scalar_mul(negm_qp, stats_qp[:, :, 0],
                                                -1.0)
                    nc.scalar.activation(std_qp, stats_qp[:, :, 1], AF.Sqrt,
                                         bias=eps_p)
                    nc.vector.reciprocal(rstd_qp, std_qp)

                    augR = ph1.tile([2, L], f16, bufs=1)
                    rbc = ph1.tile([P, L], f32, bufs=1)
                    # scatter [P, 8] partition-layout stats to q-ordered DRAM
                    # (element (p, qt) -> flat q = qt*128+p), then read back
                    negm16 = ph1.tile([P, 8], f16, bufs=1)
                    nc.vector.tensor_copy(negm16, negm_qp)
                    s_negm = dramp.tile([L], f16)
                    s_std = dramp.tile([L], f16)
                    s_rstd = dramp.tile([L], f32)
                    nc.sync.dma_start(
                        s_negm[None, :].rearrange("o (t p) -> o p t", p=P),
                        negm16)
                    nc.sync.dma_start(
                        s_std[None, :].rearrange("o (t p) -> o p t", p=P),
                        std_qp)
                    nc.sync.dma_start(
                        s_rstd[None, :].rearrange("o (t p) -> o p t", p=P),
                        rstd_qp)
                    nc.sync.dma_start(augR[0:1, :], s_negm[None, :])
                    nc.sync.dma_start(augR[1:2, :], s_std[None, :])
                    nc.sync.dma_start(rbc, s_rstd[None, :].to_broadcast((P, L)))

                with nc.named_scope("proj_qk"):
                    for hp in range(2):
                        hpc = slice(hp * P, (hp + 1) * P)
                        for qb in range(2):
                            qs = slice(qb * 512, (qb + 1) * 512)
                            kps = ph2ps.tile([P, 512], f32, tag="proj")
                            for c in range(DC):
                                nc.tensor.matmul(
                                    kps, wk_sb[:, c, hpc], qt_sb[:, c, qs],
                                    start=(c == 0), stop=(c == DC - 1))
                            nc.vector.tensor_scalar_add(
                                KaugT[0:64, 2 * hp, qs], kps[0:64, :],
                                kvb_part[0:64, hp:hp + 1])
                            nc.vector.tensor_scalar_add(
                                KaugT[0:64, 2 * hp + 1, qs], kps[64:P, :],
                                kvb_part[64:P, hp:hp + 1])

                            qps = ph2ps.tile([P, 512], f32, tag="proj")
                            for c in range(DC):
                                nc.tensor.matmul(
                                    qps, wq_sb[:, c, hpc], qt_sb[:, c, qs],
                                    start=(c == 0), stop=False)
                            nc.tensor.matmul(
                                qps, qaug_sb[0:2, hpc], augR[0:2, qs],
                                start=False, stop=True)
                            nc.vector.tensor_mul(QaugT[0:64, 2 * hp, qs],
                                                 qps[0:64, :], rbc[0:64, qs])
                            nc.vector.tensor_mul(QaugT[0:64, 2 * hp + 1, qs],
                                                 qps[64:P, :], rbc[64:P, qs])

                with nc.named_scope("proj_v"):
                    for kc in range(KC):
                        ks = slice(kc * P, (kc + 1) * P)
                        vps = ph2ps.tile([P, HD], f32, tag="vproj")
                        for c in range(DC):
                            nc.tensor.matmul(
                                vps, qt_sb[:, c, ks], wv_sb[:, c, :],
                                start=(c == 0), stop=False)
                        nc.tensor.matmul(
                            vps, ones_row[0:1, 0:P], bv_sb[0:1, :],
                            start=False, stop=True)
                        nc.vector.tensor_copy(V_sb[:, kc, :], vps)

            # ---------------- phase 3: attention main loop -------------------
            with ExitStack() as mctx:
                mn = mctx.enter_context(tc.tile_pool(name="mn", bufs=3))
                biasp = mctx.enter_context(tc.tile_pool(name="biasp", bufs=6))

                mps = mctx.enter_context(
                    tc.tile_pool(name="mps", bufs=1, space="PSUM"))

                aT = None
                for h in range(NH):
                    for qt in range(8):
                        qs128 = slice(qt * P, (qt + 1) * P)
                        qb, qi = divmod(qt, 4)

                        bias_sb = biasp.tile([P, L], f16, tag="bias")
                        nc.sync.dma_start(bias_sb, pos[h, qs128, :])
                        nc.gpsimd.dma_start(bias_sb, postag[h, qs128, :],
                                            accum_op=ALU.add)

                        qk = mps.tile([P, L], f32, tag="qk", bufs=2)
                        for s2 in range(2):
                            ss = slice(s2 * 512, (s2 + 1) * 512)
                            nc.tensor.matmul(
                                qk[:, ss], QaugT[0:65, h, qs128],
                                KaugT[0:65, h, ss], start=True, stop=True)

                        u = mn.tile([P, L], f32, tag="u")
                        nc.vector.scalar_tensor_tensor(
                            u, qk, -80.0, bias_sb,
                            op0=ALU.max, op1=ALU.add)

                        attn16 = mn.tile([P, L], f16, tag="attn")
                        rowsum = mn.tile([P, 1], f32, tag="rs")
                        nc.scalar.activation(attn16, u, AF.Exp,
                                             accum_out=rowsum)
                        rrec = mn.tile([P, 1], f32, tag="rr")
                        nc.vector.reciprocal(rrec, rowsum)
                        nc.vector.tensor_scalar_mul(attn16, attn16, rrec)
                        nc.scalar.dma_start(attn_out[h, qs128, :], attn16)

                        if qi == 0:
                            aT = mn.tile([P, KC, 512], f16, tag="aT", bufs=2)
                        for half in range(2):
                            t4 = mps.tile([P, 4, P], f32, tag="t4", bufs=2)
                            for j in range(4):
                                c = half * 4 + j
                                # transpose as a REGULAR matmul (attn.T @ I):
                                # counts as PE activity, keeps HAM at 2.4GHz
                                nc.tensor.matmul(
                                    t4[:, j, :],
                                    attn16[:, c * P:(c + 1) * P], ident16,
                                    start=True, stop=True)
                            nc.vector.tensor_copy(
                                aT[:, half * 4:half * 4 + 4,
                                   qi * P:(qi + 1) * P], t4)

                        if qi == 3:
                            qsb = slice(qb * 512, (qb + 1) * 512)
                            otps = mps.tile([64, 512], f32, tag="ot", bufs=1)
                            for c in range(KC):
                                nc.tensor.matmul(
                                    otps, V_sb[:, c, h * dk:(h + 1) * dk],
                                    aT[:, c, :],
                                    start=(c == 0), stop=(c == KC - 1))
                            hb = (h % 2) * 64
                            nc.vector.tensor_copy(
                                OT128[hb:hb + 64, h // 2, qsb], otps)

                            if h == NH - 1:
                                with nc.named_scope("out_proj"):
                                    for qc in range(qb * 4, qb * 4 + 4):
                                        qcs = slice(qc * P, (qc + 1) * P)
                                        for dmb in range(2):
                                            ds_ = slice(dmb * 512,
                                                        (dmb + 1) * 512)
                                            ops_ = mps.tile([P, 512], f32,
                                                            tag="op", bufs=1)
                                            for c in range(2):
                                                nc.tensor.matmul(
                                                    ops_,
                                                    OT128[:, c, qcs],
                                                    wo_sb[:, c, ds_],
                                                    start=(c == 0),
                                                    stop=(c == 1))
                                            osb = mn.tile([P, 512], f16,
                                                          tag="osb")
                                            nc.vector.tensor_copy(osb, ops_)
                                            nc.sync.dma_start(
                                                out_part[qcs, ds_], osb)
    nc.compile()
    return nc


def _get_nc():
    global _BUILT
    if _BUILT is None:
        _BUILT = build_nc()
    return _BUILT


def make_in_maps(inputs):
    """Host-side marshalling: shard the full inputs into 8 per-core maps."""
    f = np.float32
    h16 = np.float16
    q = np.asarray(inputs["query"], f)
    pos_bias = np.asarray(inputs["pos_bias"], f)
    postag_bias = np.asarray(inputs["postag_bias"], f)
    lex_mask = np.asarray(inputs["lex_mask"], f)
    mask = np.asarray(inputs["mask"])
    Wq = np.asarray(inputs["Wq"], f)
    bq = np.asarray(inputs["bq"], f)
    Wk = np.asarray(inputs["Wk"], f)
    bk = np.asarray(inputs["bk"], f)
    Wv = np.asarray(inputs["Wv"], f)
    bv = np.asarray(inputs["bv"], f)
    Wo = np.asarray(inputs["Wo"], f)
    gamma = np.asarray(inputs["gamma"], f)
    beta = np.asarray(inputs["beta"], f)

    WqT_g = (Wq.T * gamma[:, None]) / SCALE          # [d, out], folds LN gain
    bq_eff = (bq + beta @ Wq.T) / SCALE              # folds LN shift
    colsum = WqT_g.sum(axis=0)                       # mean-correction weights
    WkT = np.ascontiguousarray(Wk.T)
    WvT = np.ascontiguousarray(Wv.T)
    WoT = np.ascontiguousarray(Wo.T)

    in_maps = []
    for core in range(NCORES):
        b = core // 4
        g = core % 4
        h0 = g * NH
        cols = slice(g * HD, (g + 1) * HD)
        lexm_b = np.where(mask[b] == 0, NEG, lex_mask[b]).astype(f)
        in_maps.append({
            "qT": np.ascontiguousarray(q[b].T.astype(h16)),
            "qrow": np.ascontiguousarray(q[b].astype(h16)),
            "pos": np.ascontiguousarray(pos_bias[h0:h0 + NH].astype(h16)),
            "postag": np.ascontiguousarray(
                postag_bias[b, h0:h0 + NH].astype(h16)),
            "wq": np.ascontiguousarray(WqT_g[:, cols].astype(h16)),
            "wk": np.ascontiguousarray(WkT[:, cols].astype(h16)),
            "wv": np.ascontiguousarray(WvT[:, cols].astype(h16)),
            "wo": np.ascontiguousarray(WoT[cols, :].astype(h16)),
            "qaug_w": np.ascontiguousarray(
                np.stack([colsum[cols], bq_eff[cols]]).astype(h16)),
            "kv_b": np.ascontiguousarray(
                np.stack([bk[cols], bv[cols]]).astype(h16)),
            "lexm": np.ascontiguousarray(lexm_b[None, :].astype(h16)),
            "onesv": np.ones((1, L), h16),
        })
    return in_maps


def gather_outputs(inputs, results):
    f = np.float32
    q = np.asarray(inputs["query"], f)
    bo = np.asarray(inputs["bo"], f)
    attn = np.empty((B, H, L, L), f)
    out = np.empty((B, L, D), f)
    for core in range(NCORES):
        b = core // 4
        h0 = (core % 4) * NH
        attn[b, h0:h0 + NH] = results[core]["attn_out"].astype(f)
    for b in range(B):
        acc = results[b * 4]["out_part"].astype(f).copy()
        for c in range(b * 4 + 1, b * 4 + 4):
            acc += results[c]["out_part"]
        out[b] = acc + bo[None, :] + q[b]
    return out, attn


def run(inputs, **spmd_kwargs):
    """Run on hardware; returns ((out, attn), BassKernelResults)."""
    from concourse.bass_utils import run_bass_kernel_spmd
    nc = _get_nc()
    in_maps = make_in_maps(inputs)
    res = run_bass_kernel_spmd(nc, in_maps, core_ids=list(range(NCORES)),
                               **spmd_kwargs)
    return gather_outputs(inputs, res.results), res


def kernel(**inputs):
    outputs, _ = run(inputs)
    return outputs
